# revision 20
# baseline (speedup 1.0000x reference)
"""Trainium2 Bass kernel for the equivariant structure-denoising module.

Computation per node n:
    vec        = x[n, 1:4]                      # [3, 128]
    vec_local  = einsum('cf,ck->fk', vec, R)    # [128, 3]
    vec_norm   = sqrt(sum_c vec^2 + 1e-4)       # [128]
    a          = concat([x[n,0], vec_local.flat, vec_norm, extra[n]])   # [1152]
    h          = gelu(a @ W1 + b1)              # [1024], exact erf gelu
    y          = (h @ W2 + b2).reshape(4, 128)
    out        = concat([y[0:1], R @ y[1:4]])   # [4, 128]

Strategy (8 NeuronCores, data-parallel over nodes):
  - pad N 100000 -> 102400, shard 12800 nodes/core, 25 blocks of 512 nodes
  - activations feature-major A^T [1152, 512]:
      * x0 and extra chunks are pre-transposed on the host, so they DMA
        straight into A^T with contiguous 2KB lines (no on-chip work)
      * rotated vec chunks: DVE applies per-node rotations in node-major
        layout (R entries are per-partition scalars), PE transposes 128x128
      * norm chunk: squares/sums on GpSimd (otherwise idle), one batched
        sqrt per block on ScalarE (minimizes ACT table swaps), PE transpose
  - MLP1: lhsT = W1 tile (stationary), rhs = A^T chunk -> PSUM H^T chunk,
    GELU+bias on ScalarE PSUM->SBUF
  - MLP2: lhsT = H^T chunk (stationary), rhs = W2 tile -> PSUM holds Y in
    node-major layout; bias-add + output rotation on DVE, contiguous DMA out
  - matmuls run in float32r (full PE rate at moving dim 512, ~TF32 precision)
"""

import os
import sys

for _p in ("/opt/trn_rl_repo",):
    if _p not in sys.path and os.path.isdir(_p):
        sys.path.append(_p)

import numpy as np

import concourse.bacc as bacc
import concourse.mybir as mybir
import concourse.tile as tile
from concourse.bass_utils import run_bass_kernel_spmd
from concourse.masks import make_identity

F32 = mybir.dt.float32
F32R = mybir.dt.float32r

N_FULL = 100_000
N_CORES = 8
FIBER = 128
EXTRA = 512
HIDDEN = 1024
IN_DIM = FIBER * 5 + EXTRA   # 1152
OUT_DIM = FIBER * 4          # 512
EPS = 1e-4

BLK = 512                    # nodes per block (PSUM bank = 512 fp32)
P = 128
NSUB = BLK // P              # 4 subtiles of 128 nodes
N_SHARD = 12_800             # nodes per core (25 blocks)
N_PAD = N_SHARD * N_CORES    # 102400
K_CH = IN_DIM // P           # 9 input chunks
H_CH = HIDDEN // P           # 8 hidden chunks


def build_nc(nblk=N_SHARD // BLK, use_f32r=True):
    """Emit the per-core Bass program for nblk blocks of 512 nodes."""
    nshard = nblk * BLK
    nc = bacc.Bacc(None, target_bir_lowering=False)
    mm_dt = F32R if use_f32r else F32

    xv = nc.dram_tensor("xv", [nshard, 3 * FIBER], F32, kind="ExternalInput")
    x0t = nc.dram_tensor("x0t", [P, nshard], mm_dt, kind="ExternalInput")
    et = nc.dram_tensor("et", [EXTRA, nshard], mm_dt, kind="ExternalInput")
    rs = nc.dram_tensor("rs", [nblk * P, NSUB * 16], F32, kind="ExternalInput")
    w1 = nc.dram_tensor("w1", [IN_DIM, HIDDEN], mm_dt, kind="ExternalInput")
    w2 = nc.dram_tensor("w2", [HIDDEN, OUT_DIM], mm_dt, kind="ExternalInput")
    b1r = nc.dram_tensor("b1r", [P, H_CH], F32, kind="ExternalInput")
    b2b = nc.dram_tensor("b2b", [P, OUT_DIM], F32, kind="ExternalInput")
    outs = nc.dram_tensor("out", [nshard, 4 * FIBER], F32, kind="ExternalOutput")

    mult = mybir.AluOpType.mult
    add = mybir.AluOpType.add

    with tile.TileContext(nc) as tc:
        with (
            tc.tile_pool(name="consts", bufs=1) as consts,
            tc.tile_pool(name="at", bufs=2) as at_pool,
            tc.tile_pool(name="hsb", bufs=2) as h_pool,
            tc.tile_pool(name="xin", bufs=8) as x_pool,
            tc.tile_pool(name="rin", bufs=3) as r_pool,
            tc.tile_pool(name="vtmp", bufs=3) as v_pool,
            tc.tile_pool(name="vlp", bufs=6) as vl_pool,
            tc.tile_pool(name="nrm", bufs=2) as n_pool,
            tc.tile_pool(name="ysb", bufs=4) as y_pool,
            tc.tile_pool(name="osb", bufs=4) as o_pool,
            tc.tile_pool(name="tpp", bufs=2, space="PSUM") as tp_psum,
            tc.tile_pool(name="hpp", bufs=3, space="PSUM") as h_psum,
            tc.tile_pool(name="ypp", bufs=3, space="PSUM") as y_psum,
        ):
            identity_f = consts.tile([P, P], F32)
            make_identity(nc, identity_f)
            # f32r copy of the identity so transposes run at 1.5 cycles/row
            # (DVE copy rounds to f32r, satisfying the verifier)
            identity = consts.tile([P, P], mm_dt)
            nc.vector.tensor_copy(identity, identity_f)
            eps_sb = consts.tile([P, 1], F32)
            nc.vector.memset(eps_sb, EPS)

            w1_sb = []
            for k in range(K_CH):
                t = consts.tile([P, HIDDEN], mm_dt, name=f"w1_{k}", tag=f"w1_{k}")
                nc.scalar.dma_start(out=t, in_=w1[k * P:(k + 1) * P, :])
                w1_sb.append(t)
            w2_sb = []
            for j in range(H_CH):
                t = consts.tile([P, OUT_DIM], mm_dt, name=f"w2_{j}", tag=f"w2_{j}")
                nc.gpsimd.dma_start(out=t, in_=w2[j * P:(j + 1) * P, :])
                w2_sb.append(t)
            b1_sb = consts.tile([P, H_CH], F32)
            nc.gpsimd.dma_start(out=b1_sb, in_=b1r[:, :])
            b2_sb = consts.tile([P, OUT_DIM], F32)
            nc.gpsimd.dma_start(out=b2_sb, in_=b2b[:, :])

            at_blocks = {}
            r_blocks = {}

            vls_blocks = {}
            nsq_blocks = {}

            def emit_prep_a(b):
                n0 = b * BLK
                at = [at_pool.tile([P, BLK], mm_dt, name=f"at_{k}", tag=f"at_{k}")
                      for k in range(K_CH)]
                at_blocks[b] = at
                r_sb = r_pool.tile([P, NSUB * 16], F32, name="r_sb", tag="r_sb")
                r_blocks[b] = r_sb
                nc.sync.dma_start(out=r_sb, in_=rs[b * P:(b + 1) * P, :])

                # x0 and extra chunks: straight DMA from host-transposed DRAM
                nc.sync.dma_start(out=at[0], in_=x0t[:, n0:n0 + BLK])
                for t in range(4):
                    nc.sync.dma_start(
                        out=at[5 + t], in_=et[t * P:(t + 1) * P, n0:n0 + BLK])

                nsq_blk = n_pool.tile([P, BLK], F32, name="nsq", tag="nsq")
                nsq_blocks[b] = nsq_blk

                vls_all = []
                for i in range(NSUB):
                    base = n0 + i * P
                    x_sb = x_pool.tile([P, 3 * FIBER], F32, name="x_sb", tag="x_sb")
                    nc.sync.dma_start(out=x_sb, in_=xv[base:base + P, :])

                    def rsc(c, k):
                        col = i * 16 + c * 3 + k
                        return r_sb[:, col:col + 1]

                    v0 = x_sb[:, 0 * P:1 * P]
                    v1 = x_sb[:, 1 * P:2 * P]
                    v2 = x_sb[:, 2 * P:3 * P]

                    # vec_local_k = v0*R[0,k] + v1*R[1,k] + v2*R[2,k] (DVE)
                    vls = []
                    for k in range(3):
                        ta = v_pool.tile([P, P], F32, name="rot_a", tag="rot_a")
                        tb = v_pool.tile([P, P], F32, name="rot_b", tag="rot_b")
                        vl = vl_pool.tile([P, P], mm_dt, name=f"vl_{k}", tag=f"vl_{k}")
                        nc.vector.tensor_scalar_mul(ta, v0, rsc(0, k))
                        nc.vector.scalar_tensor_tensor(
                            tb, v1, rsc(1, k), ta, op0=mult, op1=add)
                        nc.vector.scalar_tensor_tensor(
                            vl, v2, rsc(2, k), tb, op0=mult, op1=add)
                        vls.append(vl)
                    vls_all.append(vls)

                    # nsq slice = v0^2 + v1^2 + v2^2 (GpSimd; otherwise idle)
                    nslice = nsq_blk[:, i * P:(i + 1) * P]
                    g1 = v_pool.tile([P, P], F32, name="gsq_a", tag="gsq_a")
                    g2 = v_pool.tile([P, P], F32, name="gsq_b", tag="gsq_b")
                    nc.gpsimd.tensor_mul(g1, v0, v0)
                    nc.gpsimd.tensor_mul(g2, v1, v1)
                    nc.gpsimd.tensor_add(g1, g1, g2)
                    nc.gpsimd.tensor_mul(g2, v2, v2)
                    nc.gpsimd.tensor_add(nslice, g1, g2)
                vls_blocks[b] = vls_all

            def emit_prep_b1(b):
                at = at_blocks[b]
                vls_all = vls_blocks.pop(b)
                # transpose vec_local tiles into A^T chunks 1..3: pack the
                # 4 subtile transposes of one chunk into one PSUM bank and
                # drain with a single wide copy
                for k in range(3):
                    pt = tp_psum.tile([P, BLK], mm_dt, name="tp", tag="tp")
                    for i in range(NSUB):
                        nc.tensor.transpose(
                            pt[:, i * P:(i + 1) * P], vls_all[i][k], identity)
                    nc.scalar.copy(at[1 + k], pt)

            def emit_prep_b2(b):
                at = at_blocks[b]
                nsq_blk = nsq_blocks.pop(b)
                vn_blk = n_pool.tile([P, BLK], mm_dt, name="vnb", tag="vnb")
                # one batched sqrt per block (single ACT table swap pair)
                nc.scalar.activation(
                    vn_blk, nsq_blk, mybir.ActivationFunctionType.Sqrt,
                    bias=eps_sb)
                pt = tp_psum.tile([P, BLK], mm_dt, name="tpn", tag="tp")
                for i in range(NSUB):
                    nc.tensor.transpose(
                        pt[:, i * P:(i + 1) * P],
                        vn_blk[:, i * P:(i + 1) * P], identity)
                nc.scalar.copy(at[4], pt)

            h_blocks = {}

            def emit_mlp1(b):
                at = at_blocks.pop(b)

                # MLP1: H^T chunk j = gelu(sum_k W1[k,j].T @ A^T[k] + b1[j])
                h_sb = []
                for j in range(H_CH):
                    hp = h_psum.tile([P, BLK], F32, name="hp", tag="hp")
                    for k in range(K_CH):
                        nc.tensor.matmul(
                            hp,
                            w1_sb[k][:, j * P:(j + 1) * P],
                            at[k],
                            start=(k == 0), stop=(k == K_CH - 1))
                    h = h_pool.tile([P, BLK], mm_dt, name=f"h_{j}", tag=f"h_{j}")
                    nc.scalar.activation(
                        h, hp, mybir.ActivationFunctionType.Gelu,
                        bias=b1_sb[:, j:j + 1])
                    h_sb.append(h)
                h_blocks[b] = h_sb

            def emit_mlp2(b):
                n0 = b * BLK
                r_sb = r_blocks.pop(b)
                h_sb = h_blocks.pop(b)

                # MLP2: Y subtile i (node-major) = sum_j H^T[j,i].T @ W2[j]
                for i in range(NSUB):
                    base = n0 + i * P
                    yp = y_psum.tile([P, OUT_DIM], F32, name="yp", tag="yp")
                    for j in range(H_CH):
                        nc.tensor.matmul(
                            yp,
                            h_sb[j][:, i * P:(i + 1) * P],
                            w2_sb[j],
                            start=(j == 0), stop=(j == H_CH - 1))
                    y_sb = y_pool.tile([P, OUT_DIM], F32, name="y_sb", tag="y_sb")
                    nc.vector.tensor_add(y_sb, yp, b2_sb)

                    def rsc(c, k):
                        col = i * 16 + c * 3 + k
                        return r_sb[:, col:col + 1]

                    o_sb = o_pool.tile([P, OUT_DIM], F32, name="o_sb", tag="o_sb")
                    nc.gpsimd.tensor_copy(o_sb[:, 0:P], y_sb[:, 0:P])
                    yv0 = y_sb[:, 1 * P:2 * P]
                    yv1 = y_sb[:, 2 * P:3 * P]
                    yv2 = y_sb[:, 3 * P:4 * P]
                    # vec_out_c = R[c,0]*yv0 + R[c,1]*yv1 + R[c,2]*yv2 (DVE)
                    for c in range(3):
                        ta = v_pool.tile([P, P], F32, name="orot_a", tag="orot_a")
                        tb = v_pool.tile([P, P], F32, name="orot_b", tag="orot_b")
                        nc.vector.tensor_scalar_mul(ta, yv0, rsc(c, 0))
                        nc.vector.scalar_tensor_tensor(
                            tb, yv1, rsc(c, 1), ta, op0=mult, op1=add)
                        nc.vector.scalar_tensor_tensor(
                            o_sb[:, (1 + c) * P:(2 + c) * P], yv2, rsc(c, 2),
                            tb, op0=mult, op1=add)
                    nc.sync.dma_start(out=outs[base:base + P, :], in_=o_sb)

            # software pipeline; emission order = Tile priority. prep_a
            # (DMA/DVE/GpSimd input work) leads by a full block; the ACT-bound
            # pieces (vl copies, sqrt, vn copy) are placed so the in-order ACT
            # and PE queues never cross-stall: gelus(b) then vl-copies(b+1)
            # then sqrt(b+1)/vn-copy(b+1) before gelus(b+1).
            emit_prep_a(0)
            emit_prep_b1(0)
            emit_prep_b2(0)
            for b in range(nblk):
                if b + 1 < nblk:
                    emit_prep_a(b + 1)
                emit_mlp1(b)
                if b + 1 < nblk:
                    emit_prep_b1(b + 1)
                emit_mlp2(b)
                if b + 1 < nblk:
                    emit_prep_b2(b + 1)

    nc.finalize()
    return nc


def prep_inputs(x, rotation_mats, extra_feats, W1, b1, W2, b2, nblk=N_SHARD // BLK):
    """Host-side shard + layout massaging. Returns per-core input maps."""
    nshard = nblk * BLK
    npad = nshard * N_CORES
    n = x.shape[0]

    x = np.asarray(x, dtype=np.float32)
    xv = np.ascontiguousarray(x[:, 1:4, :]).reshape(n, 3 * FIBER)
    x0 = np.ascontiguousarray(x[:, 0, :])                       # [n, 128]
    r = np.ascontiguousarray(np.asarray(rotation_mats, dtype=np.float32).reshape(n, 9))
    e = np.ascontiguousarray(np.asarray(extra_feats, dtype=np.float32))
    if n < npad:
        pad = npad - n
        xv = np.concatenate([xv, np.zeros((pad, 3 * FIBER), np.float32)])
        x0 = np.concatenate([x0, np.zeros((pad, FIBER), np.float32)])
        r = np.concatenate([r, np.zeros((pad, 9), np.float32)])
        e = np.concatenate([e, np.zeros((pad, EXTRA), np.float32)])

    # W1 rows permuted: our A^T row order is [x0; vl_k k-major; vn; extra],
    # reference is [x0; vl (f,k) f-major; vn; extra]
    perm = np.arange(IN_DIM)
    for k in range(3):
        perm[P + k * P + np.arange(P)] = P + np.arange(P) * 3 + k
    w1p = np.ascontiguousarray(np.asarray(W1, dtype=np.float32)[perm, :])
    w2 = np.ascontiguousarray(np.asarray(W2, dtype=np.float32))
    b1r = np.ascontiguousarray(np.asarray(b1, dtype=np.float32).reshape(H_CH, P).T)
    b2b = np.ascontiguousarray(np.tile(np.asarray(b2, dtype=np.float32), (P, 1)))

    in_maps = []
    for c in range(N_CORES):
        sl = slice(c * nshard, (c + 1) * nshard)
        rc = r[sl].reshape(nblk, NSUB, P, 9).transpose(0, 2, 1, 3)  # [nblk,P,NSUB,9]
        rc16 = np.zeros((nblk, P, NSUB, 16), np.float32)
        rc16[..., :9] = rc
        in_maps.append({
            "xv": xv[sl],
            "x0t": np.ascontiguousarray(x0[sl].T),
            "et": np.ascontiguousarray(e[sl].T),
            "rs": rc16.reshape(nblk * P, NSUB * 16),
            "w1": w1p,
            "w2": w2,
            "b1r": b1r,
            "b2b": b2b,
        })
    return in_maps


_NC_CACHE = {}


def run(x, rotation_mats, extra_feats, W1, b1, W2, b2,
        nblk=N_SHARD // BLK, trace=False, use_f32r=True):
    key = (nblk, use_f32r)
    if key not in _NC_CACHE:
        _NC_CACHE[key] = build_nc(nblk=nblk, use_f32r=use_f32r)
    nc = _NC_CACHE[key]
    in_maps = prep_inputs(x, rotation_mats, extra_feats, W1, b1, W2, b2, nblk=nblk)
    res = run_bass_kernel_spmd(nc, in_maps, list(range(N_CORES)), trace=trace)
    n = x.shape[0]
    full = np.concatenate([res.results[c]["out"] for c in range(N_CORES)], axis=0)
    out = full[:n].reshape(n, 4, FIBER)
    return out, res


def kernel(x, rotation_mats, extra_feats, W1, b1, W2, b2):
    out, _ = run(x, rotation_mats, extra_feats, W1, b1, W2, b2)
    return out


# revision 21
# speedup vs baseline: 1.0132x; 1.0132x over previous
"""Trainium2 Bass kernel for the equivariant structure-denoising module.

Computation per node n:
    vec        = x[n, 1:4]                      # [3, 128]
    vec_local  = einsum('cf,ck->fk', vec, R)    # [128, 3]
    vec_norm   = sqrt(sum_c vec^2 + 1e-4)       # [128]
    a          = concat([x[n,0], vec_local.flat, vec_norm, extra[n]])   # [1152]
    h          = gelu(a @ W1 + b1)              # [1024], exact erf gelu
    y          = (h @ W2 + b2).reshape(4, 128)
    out        = concat([y[0:1], R @ y[1:4]])   # [4, 128]

Strategy (8 NeuronCores, data-parallel over nodes):
  - pad N 100000 -> 102400, shard 12800 nodes/core, 25 blocks of 512 nodes
  - activations feature-major A^T [1152, 512]:
      * x0 and extra chunks are pre-transposed on the host, so they DMA
        straight into A^T with contiguous 2KB lines (no on-chip work)
      * rotated vec chunks: DVE applies per-node rotations in node-major
        layout (R entries are per-partition scalars), PE transposes 128x128
      * norm chunk: squares/sums on GpSimd (otherwise idle), one batched
        sqrt per block on ScalarE (minimizes ACT table swaps), PE transpose
  - MLP1: lhsT = W1 tile (stationary), rhs = A^T chunk -> PSUM H^T chunk,
    GELU+bias on ScalarE PSUM->SBUF
  - MLP2: lhsT = H^T chunk (stationary), rhs = W2 tile -> PSUM holds Y in
    node-major layout; bias-add + output rotation on DVE, contiguous DMA out
  - matmuls run in float32r (full PE rate at moving dim 512, ~TF32 precision)
"""

import os
import sys

for _p in ("/opt/trn_rl_repo",):
    if _p not in sys.path and os.path.isdir(_p):
        sys.path.append(_p)

import numpy as np

import concourse.bacc as bacc
import concourse.mybir as mybir
import concourse.tile as tile
from concourse.bass_utils import run_bass_kernel_spmd
from concourse.masks import make_identity

F32 = mybir.dt.float32
F32R = mybir.dt.float32r

N_FULL = 100_000
N_CORES = 8
FIBER = 128
EXTRA = 512
HIDDEN = 1024
IN_DIM = FIBER * 5 + EXTRA   # 1152
OUT_DIM = FIBER * 4          # 512
EPS = 1e-4

BLK = 512                    # nodes per block (PSUM bank = 512 fp32)
P = 128
NSUB = BLK // P              # 4 subtiles of 128 nodes
N_SHARD = 12_800             # nodes per core (25 blocks)
N_PAD = N_SHARD * N_CORES    # 102400
K_CH = IN_DIM // P           # 9 input chunks
H_CH = HIDDEN // P           # 8 hidden chunks


def build_nc(nblk=N_SHARD // BLK, use_f32r=True):
    """Emit the per-core Bass program for nblk blocks of 512 nodes."""
    nshard = nblk * BLK
    nc = bacc.Bacc(None, target_bir_lowering=False)
    mm_dt = F32R if use_f32r else F32

    xv = nc.dram_tensor("xv", [nshard, 3 * FIBER], F32, kind="ExternalInput")
    x0t = nc.dram_tensor("x0t", [P, nshard], mm_dt, kind="ExternalInput")
    et = nc.dram_tensor("et", [EXTRA, nshard], mm_dt, kind="ExternalInput")
    rs = nc.dram_tensor("rs", [nblk * P, NSUB * 16], F32, kind="ExternalInput")
    w1 = nc.dram_tensor("w1", [IN_DIM, HIDDEN], mm_dt, kind="ExternalInput")
    w2 = nc.dram_tensor("w2", [HIDDEN, OUT_DIM], mm_dt, kind="ExternalInput")
    b1r = nc.dram_tensor("b1r", [P, H_CH], F32, kind="ExternalInput")
    b2b = nc.dram_tensor("b2b", [P, OUT_DIM], F32, kind="ExternalInput")
    outs = nc.dram_tensor("out", [nshard, 4 * FIBER], F32, kind="ExternalOutput")

    mult = mybir.AluOpType.mult
    add = mybir.AluOpType.add

    with tile.TileContext(nc) as tc:
        with (
            tc.tile_pool(name="consts", bufs=1) as consts,
            tc.tile_pool(name="at", bufs=2) as at_pool,
            tc.tile_pool(name="hsb", bufs=2) as h_pool,
            tc.tile_pool(name="xin", bufs=8) as x_pool,
            tc.tile_pool(name="rin", bufs=3) as r_pool,
            tc.tile_pool(name="vtmp", bufs=3) as v_pool,
            tc.tile_pool(name="vlp", bufs=6) as vl_pool,
            tc.tile_pool(name="nrm", bufs=2) as n_pool,
            tc.tile_pool(name="ysb", bufs=4) as y_pool,
            tc.tile_pool(name="osb", bufs=4) as o_pool,
            tc.tile_pool(name="tpp", bufs=2, space="PSUM") as tp_psum,
            tc.tile_pool(name="hpp", bufs=3, space="PSUM") as h_psum,
            tc.tile_pool(name="ypp", bufs=3, space="PSUM") as y_psum,
        ):
            identity_f = consts.tile([P, P], F32)
            make_identity(nc, identity_f)
            # f32r copy of the identity so transposes run at 1.5 cycles/row
            # (DVE copy rounds to f32r, satisfying the verifier)
            identity = consts.tile([P, P], mm_dt)
            nc.vector.tensor_copy(identity, identity_f)
            eps_sb = consts.tile([P, 1], F32)
            nc.vector.memset(eps_sb, EPS)

            w1_sb = []
            for k in range(K_CH):
                t = consts.tile([P, HIDDEN], mm_dt, name=f"w1_{k}", tag=f"w1_{k}")
                nc.scalar.dma_start(out=t, in_=w1[k * P:(k + 1) * P, :])
                w1_sb.append(t)
            w2_sb = []
            for j in range(H_CH):
                t = consts.tile([P, OUT_DIM], mm_dt, name=f"w2_{j}", tag=f"w2_{j}")
                nc.gpsimd.dma_start(out=t, in_=w2[j * P:(j + 1) * P, :])
                w2_sb.append(t)
            b1_sb = consts.tile([P, H_CH], F32)
            nc.gpsimd.dma_start(out=b1_sb, in_=b1r[:, :])
            b2_sb = consts.tile([P, OUT_DIM], F32)
            nc.gpsimd.dma_start(out=b2_sb, in_=b2b[:, :])

            at_blocks = {}
            r_blocks = {}

            vls_blocks = {}
            nsq_blocks = {}

            def emit_prep_a(b):
                n0 = b * BLK
                at = [at_pool.tile([P, BLK], mm_dt, name=f"at_{k}", tag=f"at_{k}")
                      for k in range(K_CH)]
                at_blocks[b] = at
                r_sb = r_pool.tile([P, NSUB * 16], F32, name="r_sb", tag="r_sb")
                r_blocks[b] = r_sb
                nc.sync.dma_start(out=r_sb, in_=rs[b * P:(b + 1) * P, :])

                # x0 and extra chunks: straight DMA from host-transposed DRAM
                nc.sync.dma_start(out=at[0], in_=x0t[:, n0:n0 + BLK])
                for t in range(4):
                    nc.sync.dma_start(
                        out=at[5 + t], in_=et[t * P:(t + 1) * P, n0:n0 + BLK])

                nsq_blk = n_pool.tile([P, BLK], F32, name="nsq", tag="nsq")
                nsq_blocks[b] = nsq_blk

                vls_all = []
                for i in range(NSUB):
                    base = n0 + i * P
                    x_sb = x_pool.tile([P, 3 * FIBER], F32, name="x_sb", tag="x_sb")
                    nc.sync.dma_start(out=x_sb, in_=xv[base:base + P, :])

                    def rsc(c, k):
                        col = i * 16 + c * 3 + k
                        return r_sb[:, col:col + 1]

                    v0 = x_sb[:, 0 * P:1 * P]
                    v1 = x_sb[:, 1 * P:2 * P]
                    v2 = x_sb[:, 2 * P:3 * P]

                    # vec_local_k = v0*R[0,k] + v1*R[1,k] + v2*R[2,k] (DVE)
                    vls = []
                    for k in range(3):
                        ta = v_pool.tile([P, P], F32, name="rot_a", tag="rot_a")
                        tb = v_pool.tile([P, P], F32, name="rot_b", tag="rot_b")
                        vl = vl_pool.tile([P, P], mm_dt, name=f"vl_{k}", tag=f"vl_{k}")
                        nc.vector.tensor_scalar_mul(ta, v0, rsc(0, k))
                        nc.vector.scalar_tensor_tensor(
                            tb, v1, rsc(1, k), ta, op0=mult, op1=add)
                        nc.vector.scalar_tensor_tensor(
                            vl, v2, rsc(2, k), tb, op0=mult, op1=add)
                        vls.append(vl)
                    vls_all.append(vls)

                    # nsq slice = v0^2 + v1^2 + v2^2 (GpSimd; otherwise idle)
                    nslice = nsq_blk[:, i * P:(i + 1) * P]
                    g1 = v_pool.tile([P, P], F32, name="gsq_a", tag="gsq_a")
                    g2 = v_pool.tile([P, P], F32, name="gsq_b", tag="gsq_b")
                    nc.gpsimd.tensor_mul(g1, v0, v0)
                    nc.gpsimd.tensor_mul(g2, v1, v1)
                    nc.gpsimd.tensor_add(g1, g1, g2)
                    nc.gpsimd.tensor_mul(g2, v2, v2)
                    nc.gpsimd.tensor_add(nslice, g1, g2)
                vls_blocks[b] = vls_all

            def emit_prep_b1(b):
                at = at_blocks[b]
                vls_all = vls_blocks.pop(b)
                # transpose vec_local tiles into A^T chunks 1..3: pack the
                # 4 subtile transposes of one chunk into one PSUM bank and
                # drain with a single wide copy
                for k in range(3):
                    pt = tp_psum.tile([P, BLK], mm_dt, name="tp", tag="tp")
                    for i in range(NSUB):
                        nc.tensor.transpose(
                            pt[:, i * P:(i + 1) * P], vls_all[i][k], identity)
                    # drain on DVE: the ACT queue is busy with this block's
                    # gelus here, and PE would stall on the 2 PSUM slots
                    nc.vector.tensor_copy(at[1 + k], pt)

            def emit_prep_b2(b):
                at = at_blocks[b]
                nsq_blk = nsq_blocks.pop(b)
                vn_blk = n_pool.tile([P, BLK], mm_dt, name="vnb", tag="vnb")
                # one batched sqrt per block (single ACT table swap pair)
                nc.scalar.activation(
                    vn_blk, nsq_blk, mybir.ActivationFunctionType.Sqrt,
                    bias=eps_sb)
                pt = tp_psum.tile([P, BLK], mm_dt, name="tpn", tag="tp")
                for i in range(NSUB):
                    nc.tensor.transpose(
                        pt[:, i * P:(i + 1) * P],
                        vn_blk[:, i * P:(i + 1) * P], identity)
                nc.scalar.copy(at[4], pt)

            h_blocks = {}

            def emit_mlp1(b):
                at = at_blocks.pop(b)

                # MLP1: H^T chunk j = gelu(sum_k W1[k,j].T @ A^T[k] + b1[j])
                h_sb = []
                for j in range(H_CH):
                    hp = h_psum.tile([P, BLK], F32, name="hp", tag="hp")
                    for k in range(K_CH):
                        nc.tensor.matmul(
                            hp,
                            w1_sb[k][:, j * P:(j + 1) * P],
                            at[k],
                            start=(k == 0), stop=(k == K_CH - 1))
                    h = h_pool.tile([P, BLK], mm_dt, name=f"h_{j}", tag=f"h_{j}")
                    nc.scalar.activation(
                        h, hp, mybir.ActivationFunctionType.Gelu,
                        bias=b1_sb[:, j:j + 1])
                    h_sb.append(h)
                h_blocks[b] = h_sb

            def emit_mlp2(b):
                n0 = b * BLK
                r_sb = r_blocks.pop(b)
                h_sb = h_blocks.pop(b)

                # MLP2: Y subtile i (node-major) = sum_j H^T[j,i].T @ W2[j]
                for i in range(NSUB):
                    base = n0 + i * P
                    yp = y_psum.tile([P, OUT_DIM], F32, name="yp", tag="yp")
                    for j in range(H_CH):
                        nc.tensor.matmul(
                            yp,
                            h_sb[j][:, i * P:(i + 1) * P],
                            w2_sb[j],
                            start=(j == 0), stop=(j == H_CH - 1))
                    y_sb = y_pool.tile([P, OUT_DIM], F32, name="y_sb", tag="y_sb")
                    nc.vector.tensor_add(y_sb, yp, b2_sb)

                    def rsc(c, k):
                        col = i * 16 + c * 3 + k
                        return r_sb[:, col:col + 1]

                    o_sb = o_pool.tile([P, OUT_DIM], F32, name="o_sb", tag="o_sb")
                    nc.gpsimd.tensor_copy(o_sb[:, 0:P], y_sb[:, 0:P])
                    yv0 = y_sb[:, 1 * P:2 * P]
                    yv1 = y_sb[:, 2 * P:3 * P]
                    yv2 = y_sb[:, 3 * P:4 * P]
                    # vec_out_c = R[c,0]*yv0 + R[c,1]*yv1 + R[c,2]*yv2 (DVE)
                    for c in range(3):
                        ta = v_pool.tile([P, P], F32, name="orot_a", tag="orot_a")
                        tb = v_pool.tile([P, P], F32, name="orot_b", tag="orot_b")
                        nc.vector.tensor_scalar_mul(ta, yv0, rsc(c, 0))
                        nc.vector.scalar_tensor_tensor(
                            tb, yv1, rsc(c, 1), ta, op0=mult, op1=add)
                        nc.vector.scalar_tensor_tensor(
                            o_sb[:, (1 + c) * P:(2 + c) * P], yv2, rsc(c, 2),
                            tb, op0=mult, op1=add)
                    nc.sync.dma_start(out=outs[base:base + P, :], in_=o_sb)

            # software pipeline; emission order = Tile priority. prep_a
            # (DMA/DVE/GpSimd input work) leads by a full block; the ACT-bound
            # pieces (vl copies, sqrt, vn copy) are placed so the in-order ACT
            # and PE queues never cross-stall: gelus(b) then vl-copies(b+1)
            # then sqrt(b+1)/vn-copy(b+1) before gelus(b+1).
            emit_prep_a(0)
            emit_prep_b1(0)
            emit_prep_b2(0)
            for b in range(nblk):
                if b + 1 < nblk:
                    emit_prep_a(b + 1)
                emit_mlp1(b)
                if b + 1 < nblk:
                    emit_prep_b1(b + 1)
                emit_mlp2(b)
                if b + 1 < nblk:
                    emit_prep_b2(b + 1)

    nc.finalize()
    return nc


def prep_inputs(x, rotation_mats, extra_feats, W1, b1, W2, b2, nblk=N_SHARD // BLK):
    """Host-side shard + layout massaging. Returns per-core input maps."""
    nshard = nblk * BLK
    npad = nshard * N_CORES
    n = x.shape[0]

    x = np.asarray(x, dtype=np.float32)
    xv = np.ascontiguousarray(x[:, 1:4, :]).reshape(n, 3 * FIBER)
    x0 = np.ascontiguousarray(x[:, 0, :])                       # [n, 128]
    r = np.ascontiguousarray(np.asarray(rotation_mats, dtype=np.float32).reshape(n, 9))
    e = np.ascontiguousarray(np.asarray(extra_feats, dtype=np.float32))
    if n < npad:
        pad = npad - n
        xv = np.concatenate([xv, np.zeros((pad, 3 * FIBER), np.float32)])
        x0 = np.concatenate([x0, np.zeros((pad, FIBER), np.float32)])
        r = np.concatenate([r, np.zeros((pad, 9), np.float32)])
        e = np.concatenate([e, np.zeros((pad, EXTRA), np.float32)])

    # W1 rows permuted: our A^T row order is [x0; vl_k k-major; vn; extra],
    # reference is [x0; vl (f,k) f-major; vn; extra]
    perm = np.arange(IN_DIM)
    for k in range(3):
        perm[P + k * P + np.arange(P)] = P + np.arange(P) * 3 + k
    w1p = np.ascontiguousarray(np.asarray(W1, dtype=np.float32)[perm, :])
    w2 = np.ascontiguousarray(np.asarray(W2, dtype=np.float32))
    b1r = np.ascontiguousarray(np.asarray(b1, dtype=np.float32).reshape(H_CH, P).T)
    b2b = np.ascontiguousarray(np.tile(np.asarray(b2, dtype=np.float32), (P, 1)))

    in_maps = []
    for c in range(N_CORES):
        sl = slice(c * nshard, (c + 1) * nshard)
        rc = r[sl].reshape(nblk, NSUB, P, 9).transpose(0, 2, 1, 3)  # [nblk,P,NSUB,9]
        rc16 = np.zeros((nblk, P, NSUB, 16), np.float32)
        rc16[..., :9] = rc
        in_maps.append({
            "xv": xv[sl],
            "x0t": np.ascontiguousarray(x0[sl].T),
            "et": np.ascontiguousarray(e[sl].T),
            "rs": rc16.reshape(nblk * P, NSUB * 16),
            "w1": w1p,
            "w2": w2,
            "b1r": b1r,
            "b2b": b2b,
        })
    return in_maps


_NC_CACHE = {}


def run(x, rotation_mats, extra_feats, W1, b1, W2, b2,
        nblk=N_SHARD // BLK, trace=False, use_f32r=True):
    key = (nblk, use_f32r)
    if key not in _NC_CACHE:
        _NC_CACHE[key] = build_nc(nblk=nblk, use_f32r=use_f32r)
    nc = _NC_CACHE[key]
    in_maps = prep_inputs(x, rotation_mats, extra_feats, W1, b1, W2, b2, nblk=nblk)
    res = run_bass_kernel_spmd(nc, in_maps, list(range(N_CORES)), trace=trace)
    n = x.shape[0]
    full = np.concatenate([res.results[c]["out"] for c in range(N_CORES)], axis=0)
    out = full[:n].reshape(n, 4, FIBER)
    return out, res


def kernel(x, rotation_mats, extra_feats, W1, b1, W2, b2):
    out, _ = run(x, rotation_mats, extra_feats, W1, b1, W2, b2)
    return out


# revision 22
# speedup vs baseline: 1.0197x; 1.0064x over previous
"""Trainium2 Bass kernel for the equivariant structure-denoising module.

Computation per node n:
    vec        = x[n, 1:4]                      # [3, 128]
    vec_local  = einsum('cf,ck->fk', vec, R)    # [128, 3]
    vec_norm   = sqrt(sum_c vec^2 + 1e-4)       # [128]
    a          = concat([x[n,0], vec_local.flat, vec_norm, extra[n]])   # [1152]
    h          = gelu(a @ W1 + b1)              # [1024], exact erf gelu
    y          = (h @ W2 + b2).reshape(4, 128)
    out        = concat([y[0:1], R @ y[1:4]])   # [4, 128]

Strategy (8 NeuronCores, data-parallel over nodes):
  - pad N 100000 -> 102400, shard 12800 nodes/core, 25 blocks of 512 nodes
  - activations feature-major A^T [1152, 512]:
      * x0 and extra chunks are pre-transposed on the host, so they DMA
        straight into A^T with contiguous 2KB lines (no on-chip work)
      * rotated vec chunks: DVE applies per-node rotations in node-major
        layout (R entries are per-partition scalars), PE transposes 128x128
      * norm chunk: squares/sums on GpSimd (otherwise idle), one batched
        sqrt per block on ScalarE (minimizes ACT table swaps), PE transpose
  - MLP1: lhsT = W1 tile (stationary), rhs = A^T chunk -> PSUM H^T chunk,
    GELU+bias on ScalarE PSUM->SBUF
  - MLP2: lhsT = H^T chunk (stationary), rhs = W2 tile -> PSUM holds Y in
    node-major layout; bias-add + output rotation on DVE, contiguous DMA out
  - matmuls run in float32r (full PE rate at moving dim 512, ~TF32 precision)
"""

import os
import sys

for _p in ("/opt/trn_rl_repo",):
    if _p not in sys.path and os.path.isdir(_p):
        sys.path.append(_p)

import numpy as np

import concourse.bacc as bacc
import concourse.mybir as mybir
import concourse.tile as tile
from concourse.bass_utils import run_bass_kernel_spmd
from concourse.masks import make_identity

F32 = mybir.dt.float32
F32R = mybir.dt.float32r

N_FULL = 100_000
N_CORES = 8
FIBER = 128
EXTRA = 512
HIDDEN = 1024
IN_DIM = FIBER * 5 + EXTRA   # 1152
OUT_DIM = FIBER * 4          # 512
EPS = 1e-4

BLK = 512                    # nodes per block (PSUM bank = 512 fp32)
P = 128
NSUB = BLK // P              # 4 subtiles of 128 nodes
N_SHARD = 12_800             # nodes per core (25 blocks)
N_PAD = N_SHARD * N_CORES    # 102400
K_CH = IN_DIM // P           # 9 input chunks
H_CH = HIDDEN // P           # 8 hidden chunks


def build_nc(nblk=N_SHARD // BLK, use_f32r=True):
    """Emit the per-core Bass program for nblk blocks of 512 nodes."""
    nshard = nblk * BLK
    nc = bacc.Bacc(None, target_bir_lowering=False)
    mm_dt = F32R if use_f32r else F32

    xv = nc.dram_tensor("xv", [nshard, 3 * FIBER], F32, kind="ExternalInput")
    x0t = nc.dram_tensor("x0t", [P, nshard], mm_dt, kind="ExternalInput")
    et = nc.dram_tensor("et", [EXTRA, nshard], mm_dt, kind="ExternalInput")
    rs = nc.dram_tensor("rs", [nblk * P, NSUB * 16], F32, kind="ExternalInput")
    w1 = nc.dram_tensor("w1", [IN_DIM, HIDDEN], mm_dt, kind="ExternalInput")
    w2 = nc.dram_tensor("w2", [HIDDEN, OUT_DIM], mm_dt, kind="ExternalInput")
    b1r = nc.dram_tensor("b1r", [P, H_CH], F32, kind="ExternalInput")
    b2b = nc.dram_tensor("b2b", [P, OUT_DIM], F32, kind="ExternalInput")
    outs = nc.dram_tensor("out", [nshard, 4 * FIBER], F32, kind="ExternalOutput")

    mult = mybir.AluOpType.mult
    add = mybir.AluOpType.add

    with tile.TileContext(nc) as tc:
        with (
            tc.tile_pool(name="consts", bufs=1) as consts,
            tc.tile_pool(name="at", bufs=2) as at_pool,
            tc.tile_pool(name="hsb", bufs=2) as h_pool,
            tc.tile_pool(name="xin", bufs=8) as x_pool,
            tc.tile_pool(name="rin", bufs=3) as r_pool,
            tc.tile_pool(name="vtmp", bufs=3) as v_pool,
            tc.tile_pool(name="vlp", bufs=6) as vl_pool,
            tc.tile_pool(name="nrm", bufs=2) as n_pool,
            tc.tile_pool(name="ysb", bufs=4) as y_pool,
            tc.tile_pool(name="osb", bufs=4) as o_pool,
            tc.tile_pool(name="tpp", bufs=3, space="PSUM") as tp_psum,
            tc.tile_pool(name="hpp", bufs=3, space="PSUM") as h_psum,
            tc.tile_pool(name="ypp", bufs=2, space="PSUM") as y_psum,
        ):
            identity_f = consts.tile([P, P], F32)
            make_identity(nc, identity_f)
            # f32r copy of the identity so transposes run at 1.5 cycles/row
            # (DVE copy rounds to f32r, satisfying the verifier)
            identity = consts.tile([P, P], mm_dt)
            nc.vector.tensor_copy(identity, identity_f)
            eps_sb = consts.tile([P, 1], F32)
            nc.vector.memset(eps_sb, EPS)

            w1_sb = []
            for k in range(K_CH):
                t = consts.tile([P, HIDDEN], mm_dt, name=f"w1_{k}", tag=f"w1_{k}")
                nc.scalar.dma_start(out=t, in_=w1[k * P:(k + 1) * P, :])
                w1_sb.append(t)
            w2_sb = []
            for j in range(H_CH):
                t = consts.tile([P, OUT_DIM], mm_dt, name=f"w2_{j}", tag=f"w2_{j}")
                nc.gpsimd.dma_start(out=t, in_=w2[j * P:(j + 1) * P, :])
                w2_sb.append(t)
            b1_sb = consts.tile([P, H_CH], F32)
            nc.gpsimd.dma_start(out=b1_sb, in_=b1r[:, :])
            b2_sb = consts.tile([P, OUT_DIM], F32)
            nc.gpsimd.dma_start(out=b2_sb, in_=b2b[:, :])

            at_blocks = {}
            r_blocks = {}

            vls_blocks = {}
            nsq_blocks = {}

            def emit_prep_a(b):
                n0 = b * BLK
                at = [at_pool.tile([P, BLK], mm_dt, name=f"at_{k}", tag=f"at_{k}")
                      for k in range(K_CH)]
                at_blocks[b] = at
                r_sb = r_pool.tile([P, NSUB * 16], F32, name="r_sb", tag="r_sb")
                r_blocks[b] = r_sb
                nc.sync.dma_start(out=r_sb, in_=rs[b * P:(b + 1) * P, :])

                # x0 and extra chunks: straight DMA from host-transposed DRAM
                nc.sync.dma_start(out=at[0], in_=x0t[:, n0:n0 + BLK])
                for t in range(4):
                    nc.sync.dma_start(
                        out=at[5 + t], in_=et[t * P:(t + 1) * P, n0:n0 + BLK])

                nsq_blk = n_pool.tile([P, BLK], F32, name="nsq", tag="nsq")
                nsq_blocks[b] = nsq_blk

                vls_all = []
                for i in range(NSUB):
                    base = n0 + i * P
                    x_sb = x_pool.tile([P, 3 * FIBER], F32, name="x_sb", tag="x_sb")
                    nc.sync.dma_start(out=x_sb, in_=xv[base:base + P, :])

                    def rsc(c, k):
                        col = i * 16 + c * 3 + k
                        return r_sb[:, col:col + 1]

                    v0 = x_sb[:, 0 * P:1 * P]
                    v1 = x_sb[:, 1 * P:2 * P]
                    v2 = x_sb[:, 2 * P:3 * P]

                    # vec_local_k = v0*R[0,k] + v1*R[1,k] + v2*R[2,k] (DVE)
                    vls = []
                    for k in range(3):
                        ta = v_pool.tile([P, P], F32, name="rot_a", tag="rot_a")
                        tb = v_pool.tile([P, P], F32, name="rot_b", tag="rot_b")
                        vl = vl_pool.tile([P, P], mm_dt, name=f"vl_{k}", tag=f"vl_{k}")
                        nc.vector.tensor_scalar_mul(ta, v0, rsc(0, k))
                        nc.vector.scalar_tensor_tensor(
                            tb, v1, rsc(1, k), ta, op0=mult, op1=add)
                        nc.vector.scalar_tensor_tensor(
                            vl, v2, rsc(2, k), tb, op0=mult, op1=add)
                        vls.append(vl)
                    vls_all.append(vls)

                    # nsq slice = v0^2 + v1^2 + v2^2 (GpSimd; otherwise idle)
                    nslice = nsq_blk[:, i * P:(i + 1) * P]
                    g1 = v_pool.tile([P, P], F32, name="gsq_a", tag="gsq_a")
                    g2 = v_pool.tile([P, P], F32, name="gsq_b", tag="gsq_b")
                    nc.gpsimd.tensor_mul(g1, v0, v0)
                    nc.gpsimd.tensor_mul(g2, v1, v1)
                    nc.gpsimd.tensor_add(g1, g1, g2)
                    nc.gpsimd.tensor_mul(g2, v2, v2)
                    nc.gpsimd.tensor_add(nslice, g1, g2)
                vls_blocks[b] = vls_all

            def emit_prep_b1(b):
                at = at_blocks[b]
                vls_all = vls_blocks.pop(b)
                # transpose vec_local tiles into A^T chunks 1..3: pack the
                # 4 subtile transposes of one chunk into one PSUM bank and
                # drain with a single wide copy
                for k in range(3):
                    pt = tp_psum.tile([P, BLK], mm_dt, name="tp", tag="tp")
                    for i in range(NSUB):
                        nc.tensor.transpose(
                            pt[:, i * P:(i + 1) * P], vls_all[i][k], identity)
                    # drain on DVE: the ACT queue is busy with this block's
                    # gelus here, and PE would stall on the 2 PSUM slots
                    nc.vector.tensor_copy(at[1 + k], pt)

            def emit_prep_b2(b):
                at = at_blocks[b]
                nsq_blk = nsq_blocks.pop(b)
                vn_blk = n_pool.tile([P, BLK], mm_dt, name="vnb", tag="vnb")
                # one batched sqrt per block (single ACT table swap pair)
                nc.scalar.activation(
                    vn_blk, nsq_blk, mybir.ActivationFunctionType.Sqrt,
                    bias=eps_sb)
                pt = tp_psum.tile([P, BLK], mm_dt, name="tpn", tag="tp")
                for i in range(NSUB):
                    nc.tensor.transpose(
                        pt[:, i * P:(i + 1) * P],
                        vn_blk[:, i * P:(i + 1) * P], identity)
                nc.scalar.copy(at[4], pt)

            h_blocks = {}

            def emit_mlp1(b):
                at = at_blocks.pop(b)

                # MLP1: H^T chunk j = gelu(sum_k W1[k,j].T @ A^T[k] + b1[j])
                h_sb = []
                for j in range(H_CH):
                    hp = h_psum.tile([P, BLK], F32, name="hp", tag="hp")
                    for k in range(K_CH):
                        nc.tensor.matmul(
                            hp,
                            w1_sb[k][:, j * P:(j + 1) * P],
                            at[k],
                            start=(k == 0), stop=(k == K_CH - 1))
                    h = h_pool.tile([P, BLK], mm_dt, name=f"h_{j}", tag=f"h_{j}")
                    nc.scalar.activation(
                        h, hp, mybir.ActivationFunctionType.Gelu,
                        bias=b1_sb[:, j:j + 1])
                    h_sb.append(h)
                h_blocks[b] = h_sb

            def emit_mlp2(b):
                n0 = b * BLK
                r_sb = r_blocks.pop(b)
                h_sb = h_blocks.pop(b)

                # MLP2: Y subtile i (node-major) = sum_j H^T[j,i].T @ W2[j]
                for i in range(NSUB):
                    base = n0 + i * P
                    yp = y_psum.tile([P, OUT_DIM], F32, name="yp", tag="yp")
                    for j in range(H_CH):
                        nc.tensor.matmul(
                            yp,
                            h_sb[j][:, i * P:(i + 1) * P],
                            w2_sb[j],
                            start=(j == 0), stop=(j == H_CH - 1))
                    y_sb = y_pool.tile([P, OUT_DIM], F32, name="y_sb", tag="y_sb")
                    nc.vector.tensor_add(y_sb, yp, b2_sb)

                    def rsc(c, k):
                        col = i * 16 + c * 3 + k
                        return r_sb[:, col:col + 1]

                    o_sb = o_pool.tile([P, OUT_DIM], F32, name="o_sb", tag="o_sb")
                    nc.gpsimd.tensor_copy(o_sb[:, 0:P], y_sb[:, 0:P])
                    yv0 = y_sb[:, 1 * P:2 * P]
                    yv1 = y_sb[:, 2 * P:3 * P]
                    yv2 = y_sb[:, 3 * P:4 * P]
                    # vec_out_c = R[c,0]*yv0 + R[c,1]*yv1 + R[c,2]*yv2 (DVE)
                    for c in range(3):
                        ta = v_pool.tile([P, P], F32, name="orot_a", tag="orot_a")
                        tb = v_pool.tile([P, P], F32, name="orot_b", tag="orot_b")
                        nc.vector.tensor_scalar_mul(ta, yv0, rsc(c, 0))
                        nc.vector.scalar_tensor_tensor(
                            tb, yv1, rsc(c, 1), ta, op0=mult, op1=add)
                        nc.vector.scalar_tensor_tensor(
                            o_sb[:, (1 + c) * P:(2 + c) * P], yv2, rsc(c, 2),
                            tb, op0=mult, op1=add)
                    nc.sync.dma_start(out=outs[base:base + P, :], in_=o_sb)

            # software pipeline; emission order = Tile priority. prep_a
            # (DMA/DVE/GpSimd input work) leads by a full block; the ACT-bound
            # pieces (vl copies, sqrt, vn copy) are placed so the in-order ACT
            # and PE queues never cross-stall: gelus(b) then vl-copies(b+1)
            # then sqrt(b+1)/vn-copy(b+1) before gelus(b+1).
            emit_prep_a(0)
            emit_prep_b1(0)
            emit_prep_b2(0)
            for b in range(nblk):
                if b + 1 < nblk:
                    emit_prep_a(b + 1)
                emit_mlp1(b)
                if b + 1 < nblk:
                    emit_prep_b1(b + 1)
                emit_mlp2(b)
                if b + 1 < nblk:
                    emit_prep_b2(b + 1)

    nc.finalize()
    return nc


def prep_inputs(x, rotation_mats, extra_feats, W1, b1, W2, b2, nblk=N_SHARD // BLK):
    """Host-side shard + layout massaging. Returns per-core input maps."""
    nshard = nblk * BLK
    npad = nshard * N_CORES
    n = x.shape[0]

    x = np.asarray(x, dtype=np.float32)
    xv = np.ascontiguousarray(x[:, 1:4, :]).reshape(n, 3 * FIBER)
    x0 = np.ascontiguousarray(x[:, 0, :])                       # [n, 128]
    r = np.ascontiguousarray(np.asarray(rotation_mats, dtype=np.float32).reshape(n, 9))
    e = np.ascontiguousarray(np.asarray(extra_feats, dtype=np.float32))
    if n < npad:
        pad = npad - n
        xv = np.concatenate([xv, np.zeros((pad, 3 * FIBER), np.float32)])
        x0 = np.concatenate([x0, np.zeros((pad, FIBER), np.float32)])
        r = np.concatenate([r, np.zeros((pad, 9), np.float32)])
        e = np.concatenate([e, np.zeros((pad, EXTRA), np.float32)])

    # W1 rows permuted: our A^T row order is [x0; vl_k k-major; vn; extra],
    # reference is [x0; vl (f,k) f-major; vn; extra]
    perm = np.arange(IN_DIM)
    for k in range(3):
        perm[P + k * P + np.arange(P)] = P + np.arange(P) * 3 + k
    w1p = np.ascontiguousarray(np.asarray(W1, dtype=np.float32)[perm, :])
    w2 = np.ascontiguousarray(np.asarray(W2, dtype=np.float32))
    b1r = np.ascontiguousarray(np.asarray(b1, dtype=np.float32).reshape(H_CH, P).T)
    b2b = np.ascontiguousarray(np.tile(np.asarray(b2, dtype=np.float32), (P, 1)))

    in_maps = []
    for c in range(N_CORES):
        sl = slice(c * nshard, (c + 1) * nshard)
        rc = r[sl].reshape(nblk, NSUB, P, 9).transpose(0, 2, 1, 3)  # [nblk,P,NSUB,9]
        rc16 = np.zeros((nblk, P, NSUB, 16), np.float32)
        rc16[..., :9] = rc
        in_maps.append({
            "xv": xv[sl],
            "x0t": np.ascontiguousarray(x0[sl].T),
            "et": np.ascontiguousarray(e[sl].T),
            "rs": rc16.reshape(nblk * P, NSUB * 16),
            "w1": w1p,
            "w2": w2,
            "b1r": b1r,
            "b2b": b2b,
        })
    return in_maps


_NC_CACHE = {}


def run(x, rotation_mats, extra_feats, W1, b1, W2, b2,
        nblk=N_SHARD // BLK, trace=False, use_f32r=True):
    key = (nblk, use_f32r)
    if key not in _NC_CACHE:
        _NC_CACHE[key] = build_nc(nblk=nblk, use_f32r=use_f32r)
    nc = _NC_CACHE[key]
    in_maps = prep_inputs(x, rotation_mats, extra_feats, W1, b1, W2, b2, nblk=nblk)
    res = run_bass_kernel_spmd(nc, in_maps, list(range(N_CORES)), trace=trace)
    n = x.shape[0]
    full = np.concatenate([res.results[c]["out"] for c in range(N_CORES)], axis=0)
    out = full[:n].reshape(n, 4, FIBER)
    return out, res


def kernel(x, rotation_mats, extra_feats, W1, b1, W2, b2):
    out, _ = run(x, rotation_mats, extra_feats, W1, b1, W2, b2)
    return out
